# revision 1
# baseline (speedup 1.0000x reference)
"""GroupedQueryAttention on 8 trn2 NeuronCores.

Full shapes: q [2,8,4,2048,128], k/v [2,8,1,2048,128] -> out [2,8,4,2048,128]
softmax over S (no 1/sqrt(D) scaling; no max-subtraction needed: |scores| <~ 75
so exp() stays well inside fp32 range).

Sharding: 16 (b,h) kv pairs across 8 cores -> 2 pairs/core, each pair has
G=4 query heads sharing one K/V. Per core: 8 independent 2048x2048x128
attention heads, no communication.

Per-core kernel (all matmuls contract over the 128-partition dim):
  - K^T, Q^T prepared via PE transposes (fp32).
  - scoresT [s_tile=128, l_chunk=512] = KT.T @ QT  (float32r, 1 cyc/row)
  - ACT evicts PSUM->SBUF with Exp, output bf16.
  - PV: outT [d=128, l=512] += V_chunk.T-form matmul (lhsT=V natural, bf16)
  - softmax denominator: DVE bf16 adds (2x mode) over the 16 exp tiles
    (2 split accumulators to shorten the bf16 rounding chain), then GPSIMD
    partition_all_reduce across the s-partitions.
  - normalize outT with DVE reciprocal+mul, PE-transpose back to natural
    [l,d] layout, DMA out.
"""

import numpy as np

D = 128
L = 2048
S = 2048
G = 4  # query heads per kv head
NP = 2  # kv pairs per core
NH = NP * G  # 8 q-heads per core
LC = 512  # l chunk (matmul moving free dim)
NLC = L // LC  # 4
NST = S // 128  # 16 s tiles
NLT = L // 128  # 16 l tiles
NCORES = 8

_CACHE = {}


def _build_nc():
    import concourse.bass as bass
    import concourse.bacc as bacc
    import concourse.bass_isa as bass_isa
    import concourse.mybir as mybir
    import concourse.tile as tile
    from concourse.masks import make_identity

    f32 = mybir.dt.float32
    f32r = mybir.dt.float32r
    bf16 = mybir.dt.bfloat16
    AF = mybir.ActivationFunctionType
    ALU = mybir.AluOpType

    nc = bacc.Bacc("TRN2")
    q = nc.declare_dram_parameter("q", [NH, L, D], f32, isOutput=False)
    k = nc.declare_dram_parameter("k", [NP, S, D], f32, isOutput=False)
    v = nc.declare_dram_parameter("v", [NP, S, D], f32, isOutput=False)
    o = nc.declare_dram_parameter("o", [NH, L, D], f32, isOutput=True)

    with tile.TileContext(nc) as tc:
        with (
            tc.tile_pool(name="const", bufs=1) as constp,
            tc.tile_pool(name="kt", bufs=2) as ktp,
            tc.tile_pool(name="qt", bufs=2) as qtp,
            tc.tile_pool(name="vv", bufs=2) as vvp,
            tc.tile_pool(name="nat", bufs=4) as natp,
            tc.tile_pool(name="pe", bufs=10) as pep,
            tc.tile_pool(name="acc", bufs=16) as accp,
            tc.tile_pool(name="epi", bufs=8) as epip,
            tc.tile_pool(name="onat", bufs=12) as onatp,
            tc.tile_pool(name="psum", bufs=4, space="PSUM") as psump,
        ):
            ident = constp.tile([128, 128], f32, tag="ident")
            make_identity(nc, ident)
            nbias = constp.tile([128, 1], f32, tag="nbias")
            nc.vector.memset(nbias, -64.0)

            for pair in range(NP):
                # ---- K^T [d=128, S] via PE transposes ----
                KT = ktp.tile([128, S], f32r, tag="KT")
                for st in range(NST):
                    knat = natp.tile([128, D], f32, tag="knat")
                    nc.sync.dma_start(
                        out=knat, in_=k[pair, st * 128 : (st + 1) * 128, :]
                    )
                    pt = psump.tile([128, 128], f32, tag="ps")
                    nc.tensor.transpose(pt, knat, ident)
                    nc.vector.tensor_copy(KT[:, st * 128 : (st + 1) * 128], pt)

                # ---- V natural [s-chunk p, st, d], cast to bf16 ----
                vnat = vvp.tile([128, NST, D], f32, tag="vnat")
                nc.sync.dma_start(
                    out=vnat, in_=v[pair].rearrange("(t p) d -> p t d", p=128)
                )
                Vb = vvp.tile([128, NST, D], bf16, tag="Vb")
                nc.vector.tensor_copy(Vb, vnat)

                for g in range(G):
                    h = pair * G + g
                    # ---- Q^T [d=128, L] via PE transposes ----
                    QT = qtp.tile([128, L], f32r, tag="QT")
                    for lt in range(NLT):
                        qnat = natp.tile([128, D], f32, tag="qnat")
                        nc.sync.dma_start(
                            out=qnat, in_=q[h, lt * 128 : (lt + 1) * 128, :]
                        )
                        pt = psump.tile([128, 128], f32, tag="ps")
                        nc.tensor.transpose(pt, qnat, ident)
                        nc.vector.tensor_copy(QT[:, lt * 128 : (lt + 1) * 128], pt)

                    # out^T accumulators, one PSUM bank per l-chunk
                    po = [
                        psump.tile([128, LC], f32, tag="po", name=f"po_{h}_{lc}")
                        for lc in range(NLC)
                    ]
                    # split bf16 denominator accumulators (even/odd st)
                    acc = [
                        [
                            accp.tile(
                                [128, LC], bf16, tag="acc", name=f"acc_{h}_{lc}_{i}"
                            )
                            for i in range(2)
                        ]
                        for lc in range(NLC)
                    ]

                    for st in range(NST):
                        pss = []
                        for lc in range(NLC):
                            ps = psump.tile([128, LC], f32, tag="ps")
                            nc.tensor.matmul(
                                ps,
                                lhsT=KT[:, st * 128 : (st + 1) * 128],
                                rhs=QT[:, lc * LC : (lc + 1) * LC],
                                start=True,
                                stop=True,
                            )
                            pss.append(ps)
                        for lc in range(NLC):
                            pe = pep.tile([128, LC], bf16, tag="pe")
                            # exp(s - 64): constant shift keeps exp in fp32/bf16
                            # range (scores reach ~99; fp32 exp overflows at 88)
                            nc.scalar.activation(pe, pss[lc], AF.Exp, bias=nbias)
                            nc.tensor.matmul(
                                po[lc],
                                lhsT=Vb[:, st, :],
                                rhs=pe,
                                start=(st == 0),
                                stop=(st == NST - 1),
                            )
                            a = acc[lc][st % 2]
                            if st < 2:
                                nc.vector.tensor_copy(a, pe)
                            else:
                                nc.vector.tensor_tensor(
                                    out=a, in0=a, in1=pe, op=ALU.add
                                )

                    for lc in range(NLC):
                        den = epip.tile([128, LC], f32, tag="den")
                        nc.vector.tensor_tensor(
                            out=den, in0=acc[lc][0], in1=acc[lc][1], op=ALU.add
                        )
                        nc.gpsimd.partition_all_reduce(
                            den, den, 128, bass_isa.ReduceOp.add
                        )
                        rec = epip.tile([128, LC], f32, tag="rec")
                        nc.vector.reciprocal(rec, den)
                        oT = epip.tile([128, LC], f32, tag="oT")
                        nc.vector.tensor_tensor(
                            out=oT, in0=po[lc], in1=rec, op=ALU.mult
                        )
                        for j in range(4):
                            ptr = psump.tile([128, 128], f32, tag="ps")
                            nc.tensor.transpose(
                                ptr, oT[:, j * 128 : (j + 1) * 128], ident
                            )
                            onat = onatp.tile([128, 128], f32, tag="onat")
                            nc.vector.tensor_copy(onat, ptr)
                            lt = lc * 4 + j
                            nc.sync.dma_start(
                                out=o[h, lt * 128 : (lt + 1) * 128, :], in_=onat
                            )
    if not nc.is_finalized():
        nc.finalize()
    return nc


def _get_nc():
    if "nc" not in _CACHE:
        _CACHE["nc"] = _build_nc()
    return _CACHE["nc"]


def _run(q, k, v, trace=False, trace_kwargs=None):
    from concourse.bass_utils import run_bass_kernel_spmd

    nc = _get_nc()
    # (b,h) pair index = b*8+h; core c owns pairs 2c, 2c+1
    q6 = np.ascontiguousarray(q.reshape(16, G, L, D))
    k6 = np.ascontiguousarray(k.reshape(16, S, D))
    v6 = np.ascontiguousarray(v.reshape(16, S, D))
    in_maps = []
    for c in range(NCORES):
        sl = slice(2 * c, 2 * c + 2)
        in_maps.append(
            {
                "q": np.ascontiguousarray(q6[sl].reshape(NH, L, D)),
                "k": np.ascontiguousarray(k6[sl]),
                "v": np.ascontiguousarray(v6[sl]),
            }
        )
    kwargs = {}
    if trace:
        kwargs["trace"] = True
        if trace_kwargs:
            kwargs.update(trace_kwargs)
    res = run_bass_kernel_spmd(nc, in_maps, list(range(NCORES)), **kwargs)
    outs = [res.results[c]["o"] for c in range(NCORES)]
    full = np.concatenate(
        [o.reshape(NP, G, L, D) for o in outs], axis=0
    )  # [16, 4, L, D]
    out = full.reshape(2, 8, G, L, D).astype(np.float32)
    return out, res


def kernel(q, k, v):
    q = np.asarray(q, dtype=np.float32)
    k = np.asarray(k, dtype=np.float32)
    v = np.asarray(v, dtype=np.float32)
    out, _ = _run(q, k, v, trace=False)
    return out



# revision 2
# speedup vs baseline: 1.2683x; 1.2683x over previous
"""GroupedQueryAttention on 8 trn2 NeuronCores — pipelined fp16 transfer.

Full shapes: q [2,8,4,2048,128], k/v [2,8,1,2048,128] -> out [2,8,4,2048,128].

Wall time over the axon-tunneled PJRT link is transfer-bound (~65 MB/s each
direction, full duplex). On top of the v2 byte cuts (fp16 both ways, no
donated zero output buffers): split the work into chunks and pipeline
host cast -> upload -> exec -> download so the upload pipe runs
continuously and downloads overlap uploads.

Chunking: chunk = (kv pair within core, L-slice). Per dispatch each core
handles one kv pair's G=4 query heads over Lh = L/SPLIT_L query rows.
K/V for a pair are device_put once and shared by its L-slices.

Per-core kernel per dispatch (contract over the 128-partition dim):
  - K^T via PE transposes (fp16), V natural bf16.
  - Q^T via PE transposes per head.
  - scoresT [128, 512] = KT.T @ QT (fp16 operands, fp32 PSUM).
  - ACT evicts PSUM->SBUF with Exp(s-64), bf16.
  - PV accumulated in PSUM over s tiles; denominator via split bf16 DVE
    accumulators + GPSIMD partition_all_reduce; normalize to fp16,
    PE-transpose back, DMA out.
"""

import numpy as np

D = 128
L = 2048
S = 2048
G = 4  # query heads per kv head
NCORES = 8
SPLIT_L = 2  # L-slices per kv pair
LH = L // SPLIT_L  # query rows per dispatch
LC = 512  # l chunk (matmul moving free dim)
NLC = LH // LC
NST = S // 128  # 16 s tiles
NLT = LH // 128  # l tiles per dispatch

_CACHE = {}


def _build_nc():
    import concourse.bacc as bacc
    import concourse.bass_isa as bass_isa
    import concourse.mybir as mybir
    import concourse.tile as tile
    from concourse.masks import make_identity

    f32 = mybir.dt.float32
    f16 = mybir.dt.float16
    bf16 = mybir.dt.bfloat16
    AF = mybir.ActivationFunctionType
    ALU = mybir.AluOpType

    nc = bacc.Bacc("TRN2")
    q = nc.declare_dram_parameter("q", [G, LH, D], f16, isOutput=False)
    k = nc.declare_dram_parameter("k", [1, S, D], f16, isOutput=False)
    v = nc.declare_dram_parameter("v", [1, S, D], f16, isOutput=False)
    o = nc.declare_dram_parameter("o", [G, LH, D], f16, isOutput=True)

    with tile.TileContext(nc) as tc:
        with (
            tc.tile_pool(name="const", bufs=1) as constp,
            tc.tile_pool(name="kt", bufs=1) as ktp,
            tc.tile_pool(name="qt", bufs=2) as qtp,
            tc.tile_pool(name="vv", bufs=1) as vvp,
            tc.tile_pool(name="nat", bufs=4) as natp,
            tc.tile_pool(name="pe", bufs=10) as pep,
            tc.tile_pool(name="acc", bufs=16) as accp,
            tc.tile_pool(name="epi", bufs=8) as epip,
            tc.tile_pool(name="onat", bufs=12) as onatp,
            tc.tile_pool(name="psum", bufs=4, space="PSUM") as psump,
        ):
            ident = constp.tile([128, 128], f16, tag="ident")
            make_identity(nc, ident)
            nbias = constp.tile([128, 1], f32, tag="nbias")
            nc.vector.memset(nbias, -64.0)

            # ---- K^T [d=128, S] via PE transposes ----
            KT = ktp.tile([128, S], f16, tag="KT")
            for st in range(NST):
                knat = natp.tile([128, D], f16, tag="knat")
                nc.sync.dma_start(
                    out=knat, in_=k[0, st * 128 : (st + 1) * 128, :]
                )
                pt = psump.tile([128, 128], f16, tag="ps")
                nc.tensor.transpose(pt, knat, ident)
                nc.vector.tensor_copy(KT[:, st * 128 : (st + 1) * 128], pt)

            # ---- V natural [s-chunk p, st, d], cast to bf16 ----
            vnat = vvp.tile([128, NST, D], f16, tag="vnat")
            nc.sync.dma_start(
                out=vnat, in_=v[0].rearrange("(t p) d -> p t d", p=128)
            )
            Vb = vvp.tile([128, NST, D], bf16, tag="Vb")
            nc.vector.tensor_copy(Vb, vnat)

            for g in range(G):
                # ---- Q^T [d=128, LH] via PE transposes ----
                QT = qtp.tile([128, LH], f16, tag="QT")
                for lt in range(NLT):
                    qnat = natp.tile([128, D], f16, tag="qnat")
                    nc.sync.dma_start(
                        out=qnat, in_=q[g, lt * 128 : (lt + 1) * 128, :]
                    )
                    pt = psump.tile([128, 128], f16, tag="ps")
                    nc.tensor.transpose(pt, qnat, ident)
                    nc.vector.tensor_copy(QT[:, lt * 128 : (lt + 1) * 128], pt)

                # out^T accumulators, one PSUM bank per l-chunk
                po = [
                    psump.tile([128, LC], f32, tag="po", name=f"po_{g}_{lc}")
                    for lc in range(NLC)
                ]
                # split bf16 denominator accumulators (even/odd st)
                acc = [
                    [
                        accp.tile(
                            [128, LC], bf16, tag="acc", name=f"acc_{g}_{lc}_{i}"
                        )
                        for i in range(2)
                    ]
                    for lc in range(NLC)
                ]

                for st in range(NST):
                    pss = []
                    for lc in range(NLC):
                        ps = psump.tile([128, LC], f32, tag="ps")
                        nc.tensor.matmul(
                            ps,
                            lhsT=KT[:, st * 128 : (st + 1) * 128],
                            rhs=QT[:, lc * LC : (lc + 1) * LC],
                            start=True,
                            stop=True,
                        )
                        pss.append(ps)
                    for lc in range(NLC):
                        pe = pep.tile([128, LC], bf16, tag="pe")
                        # exp(s - 64): constant shift keeps exp in fp32/bf16
                        # range (scores reach ~99; fp32 exp overflows at 88)
                        nc.scalar.activation(pe, pss[lc], AF.Exp, bias=nbias)
                        nc.tensor.matmul(
                            po[lc],
                            lhsT=Vb[:, st, :],
                            rhs=pe,
                            start=(st == 0),
                            stop=(st == NST - 1),
                        )
                        a = acc[lc][st % 2]
                        if st < 2:
                            nc.vector.tensor_copy(a, pe)
                        else:
                            nc.vector.tensor_tensor(
                                out=a, in0=a, in1=pe, op=ALU.add
                            )

                for lc in range(NLC):
                    den = epip.tile([128, LC], f32, tag="den")
                    nc.vector.tensor_tensor(
                        out=den, in0=acc[lc][0], in1=acc[lc][1], op=ALU.add
                    )
                    nc.gpsimd.partition_all_reduce(
                        den, den, 128, bass_isa.ReduceOp.add
                    )
                    rec = epip.tile([128, LC], f32, tag="rec")
                    nc.vector.reciprocal(rec, den)
                    oT = epip.tile([128, LC], f16, tag="oT")
                    nc.vector.tensor_tensor(
                        out=oT, in0=po[lc], in1=rec, op=ALU.mult
                    )
                    for j in range(4):
                        ptr = psump.tile([128, 128], f16, tag="ps")
                        nc.tensor.transpose(
                            ptr, oT[:, j * 128 : (j + 1) * 128], ident
                        )
                        onat = onatp.tile([128, 128], f16, tag="onat")
                        nc.vector.tensor_copy(onat, ptr)
                        lt = lc * 4 + j
                        nc.sync.dma_start(
                            out=o[g, lt * 128 : (lt + 1) * 128, :], in_=onat
                        )
    if not nc.is_finalized():
        nc.finalize()
    return nc


def _get_ctx():
    if "ctx" in _CACHE:
        return _CACHE["ctx"]
    import jax
    import numpy as _np
    from jax.experimental.shard_map import shard_map
    from jax.sharding import Mesh, NamedSharding, PartitionSpec

    import concourse.bass2jax as b2j

    b2j.install_neuronx_cc_hook()
    nc = _build_nc()
    out_aval = jax.core.ShapedArray((G, LH, D), _np.float16)

    def _body(q, k, v):
        (o,) = b2j._bass_exec_p.bind(
            q,
            k,
            v,
            b2j.partition_id_tensor(),
            out_avals=(out_aval,),
            in_names=("q", "k", "v", "partition_id"),
            out_names=("o",),
            lowering_input_output_aliases=(),
            sim_require_finite=True,
            sim_require_nnan=True,
            nc=nc,
        )
        return o

    devices = jax.devices()[:NCORES]
    mesh = Mesh(np.asarray(devices), ("core",))
    P = PartitionSpec
    fn = jax.jit(
        shard_map(
            _body,
            mesh=mesh,
            in_specs=(P("core"),) * 3,
            out_specs=P("core"),
            check_rep=False,
        )
    )
    sh = NamedSharding(mesh, P("core"))
    ctx = (fn, sh, jax.device_put)
    _CACHE["ctx"] = ctx
    return ctx


def _run(q, k, v, trace=False, trace_kwargs=None):
    fn, sh, device_put = _get_ctx()
    # core c owns kv pairs 2c, 2c+1 of the 16 global (b,h) pairs.
    # chunk (p, s): pair index-within-core p, L-slice s.
    q6 = q.reshape(16, G, L, D)
    k6 = k.reshape(16, S, D)
    v6 = v.reshape(16, S, D)

    outs = {}
    kv_dev = {}
    # issue everything without blocking; transfers stream in the background
    for p in range(2):
        kd = device_put(
            k6[p::2].astype(np.float16).reshape(NCORES, S, D),
            sh,
        )
        vd = device_put(
            v6[p::2].astype(np.float16).reshape(NCORES, S, D),
            sh,
        )
        kv_dev[p] = (kd, vd)
        for s in range(SPLIT_L):
            qc = (
                q6[p::2, :, s * LH : (s + 1) * LH, :]
                .astype(np.float16)
                .reshape(NCORES * G, LH, D)
            )
            qd = device_put(qc, sh)
            oc = fn(qd, kd, vd)
            oc.copy_to_host_async()
            outs[(p, s)] = oc

    out = np.empty((16, G, L, D), np.float32)
    for p in range(2):
        for s in range(SPLIT_L):
            oc = np.asarray(outs[(p, s)])  # [NCORES*G, LH, D] fp16
            out[p::2, :, s * LH : (s + 1) * LH, :] = oc.reshape(
                NCORES, G, LH, D
            ).astype(np.float32)
    return out.reshape(2, 8, G, L, D), None


def kernel(q, k, v):
    q = np.asarray(q, dtype=np.float32)
    k = np.asarray(k, dtype=np.float32)
    v = np.asarray(v, dtype=np.float32)
    out, _ = _run(q, k, v, trace=False)
    return out
